# revision 28
# baseline (speedup 1.0000x reference)
import sys

if "/opt/trn_rl_repo" not in sys.path:
    sys.path.insert(0, "/opt/trn_rl_repo")

import numpy as np

B, HD, H, W, K = 2, 4, 128, 128, 49
KS = 7
NSP = 9
S = 64
N_CORES = 8
WQ = W // 4            # 32 columns per core
WB = 8                 # w-columns per block
NBLK = WQ // WB        # 4 blocks
G = WB * HD * NSP      # 288 (w,hd,s) groups per block
U_SZ = G * K           # 14112
AT_SZ = WB * HD * K    # 1568
ATQ = WQ * HD * K      # 6272

_cached = {}


def _build():
    import concourse.bass as bass
    import concourse.tile as tile
    from concourse import bacc, mybir

    f32 = mybir.dt.float32
    f16 = mybir.dt.float16

    nc = bacc.Bacc("TRN2", target_bir_lowering=False, debug=False, num_devices=N_CORES)
    pq = nc.dram_tensor("pq", [128, WQ * NSP * K], f16, kind="ExternalInput")
    at = nc.dram_tensor("at", [128, ATQ], f16, kind="ExternalInput")
    pi = nc.dram_tensor("pi", [128, WQ * NSP], f16, kind="ExternalInput")
    out_s = nc.dram_tensor("out", [128, ATQ], f32, kind="ExternalOutput")

    def ap(t, off, dims):
        return bass.AP(t, off, [list(d) for d in dims])

    def sap(tap, extra_off, dims):
        return bass.AP(tap.tensor, tap.offset + extra_off,
                       [list(tap.ap[0]), *[list(d) for d in dims]])

    mult = mybir.AluOpType.mult
    add = mybir.AluOpType.add
    HALF = U_SZ // 2

    with tile.TileContext(nc) as tc:
        with (
            tc.tile_pool(name="pip", bufs=1) as pip,
            tc.tile_pool(name="ep", bufs=1) as ep,
            tc.tile_pool(name="inp", bufs=2) as inp,
            tc.tile_pool(name="up", bufs=2) as up,
            tc.tile_pool(name="tp", bufs=1) as tp,
            tc.tile_pool(name="sp", bufs=2) as sp,
            tc.tile_pool(name="wp", bufs=2) as wp,
            tc.tile_pool(name="op", bufs=2) as op,
        ):
            pi_t = pip.tile([128, WQ * NSP], f16)
            at_t = ep.tile([128, ATQ], f16)
            e_t = ep.tile([128, ATQ], f16)
            nc.sync.dma_start(
                sap(at_t[:], 0, [(1, AT_SZ)]),
                ap(at, 0, [(ATQ, 128), (1, AT_SZ)]),
            )
            nc.sync.dma_start(pi_t[:], pi.ap())

            PH = WB * NSP * K // 2

            def load_p(blk):
                # split the P load across two HWDGE queues (sync + scalar)
                P4 = inp.tile([128, WB * NSP * K], f16, tag="P")
                nc.sync.dma_start(
                    sap(P4[:], 0, [(1, PH)]),
                    ap(pq, blk * WB * NSP * K, [(WQ * NSP * K, 128), (1, PH)]),
                )
                nc.scalar.dma_start(
                    sap(P4[:], PH, [(1, PH)]),
                    ap(pq, blk * WB * NSP * K + PH, [(WQ * NSP * K, 128), (1, PH)]),
                )
                return P4

            def front(blk, P4):
                # All exps run before any wgt-broadcast enters the scalar
                # queue, so later blocks' u-TTs never wait on the Act engine.
                # Block 0 in halves for a fast ramp; blocks 1-3 as one op.
                if blk == 0:
                    ECH = AT_SZ // 2
                    for c in range(2):
                        nc.scalar.activation(
                            sap(e_t[:], c * ECH, [(1, ECH)]),
                            sap(at_t[:], c * ECH, [(1, ECH)]),
                            mybir.ActivationFunctionType.Exp,
                        )
                    nc.sync.dma_start(
                        sap(at_t[:], AT_SZ, [(1, (NBLK - 1) * AT_SZ)]),
                        ap(at, AT_SZ, [(ATQ, 128), (1, (NBLK - 1) * AT_SZ)]),
                    )
                    nc.scalar.activation(
                        sap(e_t[:], AT_SZ, [(1, (NBLK - 1) * AT_SZ)]),
                        sap(at_t[:], AT_SZ, [(1, (NBLK - 1) * AT_SZ)]),
                        mybir.ActivationFunctionType.Exp,
                    )

                # u[w,hd,s,j] = P[w,s,j] * e[w,hd,j]   (fp16 2x)
                U = up.tile([128, U_SZ], f16, tag="U")
                n_u = 2 if blk == 0 else 1
                WBC = WB // n_u
                for c in range(n_u):
                    nc.vector.tensor_tensor(
                        out=sap(U[:], c * WBC * HD * NSP * K,
                                [(HD * NSP * K, WBC), (NSP * K, HD), (K, NSP), (1, K)]),
                        in0=sap(P4[:], c * WBC * NSP * K,
                                [(NSP * K, WBC), (0, HD), (K, NSP), (1, K)]),
                        in1=sap(e_t[:], blk * AT_SZ + c * WBC * HD * K,
                                [(HD * K, WBC), (K, HD), (0, NSP), (1, K)]),
                        op=mult,
                    )

                # den[w,hd,s] = sum_j u : fp16 halving tree then 4-wide reduce
                T24 = tp.tile([128, G * 24], f16, tag="T")
                nc.vector.tensor_tensor(
                    out=sap(T24[:], 0, [(24, G), (1, 24)]),
                    in0=sap(U[:], 0, [(K, G), (1, 24)]),
                    in1=sap(U[:], 24, [(K, G), (1, 24)]),
                    op=add,
                )
                nc.vector.tensor_tensor(
                    out=sap(T24[:], 0, [(24, G), (1, 12)]),
                    in0=sap(T24[:], 0, [(24, G), (1, 12)]),
                    in1=sap(T24[:], 12, [(24, G), (1, 12)]),
                    op=add,
                )
                nc.vector.tensor_tensor(
                    out=sap(T24[:], 0, [(24, G), (1, 6)]),
                    in0=sap(T24[:], 0, [(24, G), (1, 6)]),
                    in1=sap(T24[:], 6, [(24, G), (1, 6)]),
                    op=add,
                )
                D6 = sp.tile([128, G], f32, tag="D6")
                nc.vector.reduce_sum(
                    out=sap(D6[:], 0, [(1, G)]),
                    in_=sap(T24[:], 0, [(24, G), (1, 6)]),
                    axis=mybir.AxisListType.X,
                )
                # straggler u[...,48] added during den assembly (contig out)
                DEN = sp.tile([128, G], f32, tag="D")
                nc.vector.tensor_tensor(
                    out=sap(DEN[:], 0, [(1, G)]),
                    in0=sap(D6[:], 0, [(1, G)]),
                    in1=sap(U[:], 48, [(K, G)]),
                    op=add,
                )

                # wgt[w,hd,s] = pi[w,s] / den
                RCP = sp.tile([128, G], f32, tag="C")
                nc.vector.reciprocal_approx_fast(RCP[:], DEN[:])
                WT = sp.tile([128, G], f32, tag="W")
                nc.vector.tensor_tensor(
                    out=sap(WT[:], 0, [(NSP * HD, WB), (NSP, HD), (1, NSP)]),
                    in0=sap(RCP[:], 0, [(NSP * HD, WB), (NSP, HD), (1, NSP)]),
                    in1=sap(pi_t[:], blk * WB * NSP, [(NSP, WB), (0, HD), (1, NSP)]),
                    op=mult,
                )

                # broadcast wgt over j on the scalar engine, four quarters,
                # so acc can start as soon as the first quarter lands
                WR = wp.tile([128, U_SZ], f16, tag="WR")
                QTR = U_SZ // 4
                GQ = G // 4
                for q in range(4):
                    nc.scalar.copy(
                        sap(WR[:], q * QTR, [(K, GQ), (1, K)]),
                        sap(WT[:], q * GQ, [(1, GQ), (0, K)]),
                    )
                return U, WR

            def back(blk, U, WR):
                QTR = U_SZ // 4
                # acc = u * wgt (in place, fp16 2x, four quarters)
                for q in range(4):
                    nc.vector.tensor_tensor(
                        out=sap(U[:], q * QTR, [(1, QTR)]),
                        in0=sap(U[:], q * QTR, [(1, QTR)]),
                        in1=sap(WR[:], q * QTR, [(1, QTR)]),
                        op=mult,
                    )

                # sum over s: 8 -> 4 -> 2 -> 1, straggler s=8 folded in final
                eng = nc.vector
                eng.tensor_tensor(
                    out=sap(U[:], 0, [(NSP * K, WB * HD), (K, 4), (1, K)]),
                    in0=sap(U[:], 0, [(NSP * K, WB * HD), (K, 4), (1, K)]),
                    in1=sap(U[:], 4 * K, [(NSP * K, WB * HD), (K, 4), (1, K)]),
                    op=add,
                )
                eng.tensor_tensor(
                    out=sap(U[:], 0, [(NSP * K, WB * HD), (K, 2), (1, K)]),
                    in0=sap(U[:], 0, [(NSP * K, WB * HD), (K, 2), (1, K)]),
                    in1=sap(U[:], 2 * K, [(NSP * K, WB * HD), (K, 2), (1, K)]),
                    op=add,
                )
                eng.tensor_tensor(
                    out=sap(U[:], 0, [(NSP * K, WB * HD), (1, K)]),
                    in0=sap(U[:], 0, [(NSP * K, WB * HD), (1, K)]),
                    in1=sap(U[:], K, [(NSP * K, WB * HD), (1, K)]),
                    op=add,
                )
                # final add (folds straggler s=8) + fp16->fp32 casting store
                # (gpsimd SWDGE); last block in halves to overlap the tail
                O4 = op.tile([128, AT_SZ], f16, tag="O")
                n_fin = 2 if blk == NBLK - 1 else 1
                GF = WB * HD // n_fin
                for c in range(n_fin):
                    eng.tensor_tensor(
                        out=sap(O4[:], c * GF * K, [(K, GF), (1, K)]),
                        in0=sap(U[:], c * GF * NSP * K, [(NSP * K, GF), (1, K)]),
                        in1=sap(U[:], c * GF * NSP * K + 8 * K, [(NSP * K, GF), (1, K)]),
                        op=add,
                    )
                    nc.gpsimd.dma_start(
                        ap(out_s, blk * AT_SZ + c * GF * K, [(ATQ, 128), (1, GF * K)]),
                        sap(O4[:], c * GF * K, [(1, GF * K)]),
                    )

            for blk in range(NBLK):
                P4 = load_p(blk)
                U, WR = front(blk, P4)
                back(blk, U, WR)
    nc.compile()
    return nc


def _host_prep(attn, sims, sinds):
    hj = (np.clip(np.arange(H) - KS // 2, 0, H - KS)[:, None] + np.arange(KS)[None, :])
    wj = (np.clip(np.arange(W) - KS // 2, 0, W - KS)[:, None] + np.arange(KS)[None, :])
    h_idx = np.arange(H, dtype=np.intp)[:, None, None]
    w_idx = np.arange(WQ, dtype=np.intp)[None, :, None]
    in_maps = []
    for b in range(B):
        sims_b = sims[b]                                  # (S,H,W)
        SW = sims_b[:, hj, :]                             # (S,H,7,W)
        for q in range(4):
            wsl = slice(WQ * q, WQ * (q + 1))
            wq = wj[wsl]                                  # (WQ,7)
            SWq = np.ascontiguousarray(SW[:, :, :, wq])   # (S,H,7,WQ,7)
            c_idx = sinds[b][:, wsl, :].astype(np.intp)   # (H,WQ,9)
            p = SWq[c_idx, h_idx, :, w_idx, :]            # (H,WQ,9,7,7)
            p = p.reshape(H, WQ * NSP * K).astype(np.float16)
            pi = sims_b[c_idx, h_idx, w_idx + WQ * q]     # (H,WQ,9)
            pi = pi.reshape(H, WQ * NSP).astype(np.float16)
            a = attn[b][:, :, wsl, :].transpose(1, 2, 0, 3)  # (H,WQ,HD,K)
            a = np.ascontiguousarray(a.reshape(H, ATQ)).astype(np.float16)
            in_maps.append({"pq": p, "at": a, "pi": pi})
    return in_maps


def kernel(attn, sims, sinds):
    from concourse.bass_utils import run_bass_kernel_spmd

    attn = np.asarray(attn, dtype=np.float32)
    sims = np.asarray(sims, dtype=np.float32)
    sinds = np.asarray(sinds)

    if "nc" not in _cached:
        _cached["nc"] = _build()
    nc = _cached["nc"]

    in_maps = _host_prep(attn, sims, sinds)
    res = run_bass_kernel_spmd(nc, in_maps, list(range(N_CORES)))

    out = np.empty((B, HD, H, W, K), dtype=np.float32)
    for cid in range(N_CORES):
        b, q = divmod(cid, 4)
        o = res.results[cid]["out"].reshape(H, WQ, HD, K).transpose(2, 0, 1, 3)
        out[b][:, :, WQ * q:WQ * (q + 1), :] = o
    return out


# revision 29
# speedup vs baseline: 1.1886x; 1.1886x over previous
import sys

if "/opt/trn_rl_repo" not in sys.path:
    sys.path.insert(0, "/opt/trn_rl_repo")

import numpy as np

B, HD, H, W, K = 2, 4, 128, 128, 49
KS = 7
NSP = 9
S = 64
N_CORES = 8
WQ = W // 4            # 32 columns per core
WB = 8                 # w-columns per block
NBLK = WQ // WB        # 4 blocks
G = WB * HD * NSP      # 288 (w,hd,s) groups per block
U_SZ = G * K           # 14112
AT_SZ = WB * HD * K    # 1568
ATQ = WQ * HD * K      # 6272

_cached = {}


def _build():
    import concourse.bass as bass
    import concourse.tile as tile
    from concourse import bacc, mybir

    f32 = mybir.dt.float32
    f16 = mybir.dt.float16

    nc = bacc.Bacc("TRN2", target_bir_lowering=False, debug=False, num_devices=N_CORES)
    pq = nc.dram_tensor("pq", [128, WQ * NSP * K], f16, kind="ExternalInput")
    at = nc.dram_tensor("at", [128, ATQ], f16, kind="ExternalInput")
    pi = nc.dram_tensor("pi", [128, WQ * NSP], f16, kind="ExternalInput")
    out_s = nc.dram_tensor("out", [128, ATQ], f32, kind="ExternalOutput")

    def ap(t, off, dims):
        return bass.AP(t, off, [list(d) for d in dims])

    def sap(tap, extra_off, dims):
        return bass.AP(tap.tensor, tap.offset + extra_off,
                       [list(tap.ap[0]), *[list(d) for d in dims]])

    mult = mybir.AluOpType.mult
    add = mybir.AluOpType.add
    HALF = U_SZ // 2

    with tile.TileContext(nc) as tc:
        with (
            tc.tile_pool(name="pip", bufs=1) as pip,
            tc.tile_pool(name="ep", bufs=1) as ep,
            tc.tile_pool(name="inp", bufs=2) as inp,
            tc.tile_pool(name="up", bufs=2) as up,
            tc.tile_pool(name="tp", bufs=1) as tp,
            tc.tile_pool(name="sp", bufs=2) as sp,
            tc.tile_pool(name="wp", bufs=2) as wp,
            tc.tile_pool(name="op", bufs=2) as op,
        ):
            pi_t = pip.tile([128, WQ * NSP], f16)
            at_t = ep.tile([128, ATQ], f16)
            e_t = ep.tile([128, ATQ], f16)
            nc.sync.dma_start(
                sap(at_t[:], 0, [(1, AT_SZ)]),
                ap(at, 0, [(ATQ, 128), (1, AT_SZ)]),
            )
            nc.sync.dma_start(pi_t[:], pi.ap())

            PH = WB * NSP * K // 2

            def load_p(blk):
                # split the P load across two HWDGE queues (sync + scalar)
                P4 = inp.tile([128, WB * NSP * K], f16, tag="P")
                nc.sync.dma_start(
                    sap(P4[:], 0, [(1, PH)]),
                    ap(pq, blk * WB * NSP * K, [(WQ * NSP * K, 128), (1, PH)]),
                )
                nc.scalar.dma_start(
                    sap(P4[:], PH, [(1, PH)]),
                    ap(pq, blk * WB * NSP * K + PH, [(WQ * NSP * K, 128), (1, PH)]),
                )
                return P4

            def front(blk, P4):
                # attn chunk for the NEXT block while this one computes
                if blk + 1 < NBLK:
                    nc.sync.dma_start(
                        sap(at_t[:], (blk + 1) * AT_SZ, [(1, AT_SZ)]),
                        ap(at, (blk + 1) * AT_SZ, [(ATQ, 128), (1, AT_SZ)]),
                    )
                # e = exp(attn) for this block (scalar engine); block 0 runs
                # in halves so the first u-TT starts sooner after the ramp
                n_exp = 2 if blk == 0 else 1
                ECH = AT_SZ // n_exp
                for c in range(n_exp):
                    nc.scalar.activation(
                        sap(e_t[:], blk * AT_SZ + c * ECH, [(1, ECH)]),
                        sap(at_t[:], blk * AT_SZ + c * ECH, [(1, ECH)]),
                        mybir.ActivationFunctionType.Exp,
                    )

                # u[w,hd,s,j] = P[w,s,j] * e[w,hd,j]   (fp16 2x)
                U = up.tile([128, U_SZ], f16, tag="U")
                n_u = 2 if blk == 0 else 1
                WBC = WB // n_u
                for c in range(n_u):
                    nc.vector.tensor_tensor(
                        out=sap(U[:], c * WBC * HD * NSP * K,
                                [(HD * NSP * K, WBC), (NSP * K, HD), (K, NSP), (1, K)]),
                        in0=sap(P4[:], c * WBC * NSP * K,
                                [(NSP * K, WBC), (0, HD), (K, NSP), (1, K)]),
                        in1=sap(e_t[:], blk * AT_SZ + c * WBC * HD * K,
                                [(HD * K, WBC), (K, HD), (0, NSP), (1, K)]),
                        op=mult,
                    )

                # den[w,hd,s] = sum_j u : fp16 halving tree then 4-wide reduce
                T24 = tp.tile([128, G * 24], f16, tag="T")
                nc.vector.tensor_tensor(
                    out=sap(T24[:], 0, [(24, G), (1, 24)]),
                    in0=sap(U[:], 0, [(K, G), (1, 24)]),
                    in1=sap(U[:], 24, [(K, G), (1, 24)]),
                    op=add,
                )
                nc.vector.tensor_tensor(
                    out=sap(T24[:], 0, [(24, G), (1, 12)]),
                    in0=sap(T24[:], 0, [(24, G), (1, 12)]),
                    in1=sap(T24[:], 12, [(24, G), (1, 12)]),
                    op=add,
                )
                nc.vector.tensor_tensor(
                    out=sap(T24[:], 0, [(24, G), (1, 6)]),
                    in0=sap(T24[:], 0, [(24, G), (1, 6)]),
                    in1=sap(T24[:], 6, [(24, G), (1, 6)]),
                    op=add,
                )
                D6 = sp.tile([128, G], f32, tag="D6")
                nc.vector.reduce_sum(
                    out=sap(D6[:], 0, [(1, G)]),
                    in_=sap(T24[:], 0, [(24, G), (1, 6)]),
                    axis=mybir.AxisListType.X,
                )
                # straggler u[...,48] added during den assembly (contig out)
                DEN = sp.tile([128, G], f32, tag="D")
                nc.vector.tensor_tensor(
                    out=sap(DEN[:], 0, [(1, G)]),
                    in0=sap(D6[:], 0, [(1, G)]),
                    in1=sap(U[:], 48, [(K, G)]),
                    op=add,
                )

                # wgt[w,hd,s] = pi[w,s] / den
                RCP = sp.tile([128, G], f32, tag="C")
                nc.vector.reciprocal_approx_fast(RCP[:], DEN[:])
                WT = sp.tile([128, G], f32, tag="W")
                nc.vector.tensor_tensor(
                    out=sap(WT[:], 0, [(NSP * HD, WB), (NSP, HD), (1, NSP)]),
                    in0=sap(RCP[:], 0, [(NSP * HD, WB), (NSP, HD), (1, NSP)]),
                    in1=sap(pi_t[:], blk * WB * NSP, [(NSP, WB), (0, HD), (1, NSP)]),
                    op=mult,
                )

                # broadcast wgt over j on the scalar engine, four quarters,
                # so acc can start as soon as the first quarter lands
                WR = wp.tile([128, U_SZ], f16, tag="WR")
                QTR = U_SZ // 4
                GQ = G // 4
                for q in range(4):
                    nc.scalar.copy(
                        sap(WR[:], q * QTR, [(K, GQ), (1, K)]),
                        sap(WT[:], q * GQ, [(1, GQ), (0, K)]),
                    )
                return U, WR

            def back(blk, U, WR):
                QTR = U_SZ // 4
                # acc = u * wgt (in place, fp16 2x, four quarters)
                for q in range(4):
                    nc.vector.tensor_tensor(
                        out=sap(U[:], q * QTR, [(1, QTR)]),
                        in0=sap(U[:], q * QTR, [(1, QTR)]),
                        in1=sap(WR[:], q * QTR, [(1, QTR)]),
                        op=mult,
                    )

                # sum over s: 8 -> 4 -> 2 -> 1, straggler s=8 folded in final
                eng = nc.vector
                eng.tensor_tensor(
                    out=sap(U[:], 0, [(NSP * K, WB * HD), (K, 4), (1, K)]),
                    in0=sap(U[:], 0, [(NSP * K, WB * HD), (K, 4), (1, K)]),
                    in1=sap(U[:], 4 * K, [(NSP * K, WB * HD), (K, 4), (1, K)]),
                    op=add,
                )
                eng.tensor_tensor(
                    out=sap(U[:], 0, [(NSP * K, WB * HD), (K, 2), (1, K)]),
                    in0=sap(U[:], 0, [(NSP * K, WB * HD), (K, 2), (1, K)]),
                    in1=sap(U[:], 2 * K, [(NSP * K, WB * HD), (K, 2), (1, K)]),
                    op=add,
                )
                eng.tensor_tensor(
                    out=sap(U[:], 0, [(NSP * K, WB * HD), (1, K)]),
                    in0=sap(U[:], 0, [(NSP * K, WB * HD), (1, K)]),
                    in1=sap(U[:], K, [(NSP * K, WB * HD), (1, K)]),
                    op=add,
                )
                # final add (folds straggler s=8) + fp16->fp32 casting store
                # (gpsimd SWDGE); last block in halves to overlap the tail
                O4 = op.tile([128, AT_SZ], f16, tag="O")
                n_fin = 2 if blk == NBLK - 1 else 1
                GF = WB * HD // n_fin
                for c in range(n_fin):
                    eng.tensor_tensor(
                        out=sap(O4[:], c * GF * K, [(K, GF), (1, K)]),
                        in0=sap(U[:], c * GF * NSP * K, [(NSP * K, GF), (1, K)]),
                        in1=sap(U[:], c * GF * NSP * K + 8 * K, [(NSP * K, GF), (1, K)]),
                        op=add,
                    )
                    nc.gpsimd.dma_start(
                        ap(out_s, blk * AT_SZ + c * GF * K, [(ATQ, 128), (1, GF * K)]),
                        sap(O4[:], c * GF * K, [(1, GF * K)]),
                    )

            for blk in range(NBLK):
                P4 = load_p(blk)
                U, WR = front(blk, P4)
                back(blk, U, WR)
    nc.compile()
    return nc


def _host_prep(attn, sims, sinds):
    hj = (np.clip(np.arange(H) - KS // 2, 0, H - KS)[:, None] + np.arange(KS)[None, :])
    wj = (np.clip(np.arange(W) - KS // 2, 0, W - KS)[:, None] + np.arange(KS)[None, :])
    h_idx = np.arange(H, dtype=np.intp)[:, None, None]
    w_idx = np.arange(WQ, dtype=np.intp)[None, :, None]
    in_maps = []
    for b in range(B):
        sims_b = sims[b]                                  # (S,H,W)
        SW = sims_b[:, hj, :]                             # (S,H,7,W)
        for q in range(4):
            wsl = slice(WQ * q, WQ * (q + 1))
            wq = wj[wsl]                                  # (WQ,7)
            SWq = np.ascontiguousarray(SW[:, :, :, wq])   # (S,H,7,WQ,7)
            c_idx = sinds[b][:, wsl, :].astype(np.intp)   # (H,WQ,9)
            p = SWq[c_idx, h_idx, :, w_idx, :]            # (H,WQ,9,7,7)
            p = p.reshape(H, WQ * NSP * K).astype(np.float16)
            pi = sims_b[c_idx, h_idx, w_idx + WQ * q]     # (H,WQ,9)
            pi = pi.reshape(H, WQ * NSP).astype(np.float16)
            a = attn[b][:, :, wsl, :].transpose(1, 2, 0, 3)  # (H,WQ,HD,K)
            a = np.ascontiguousarray(a.reshape(H, ATQ)).astype(np.float16)
            in_maps.append({"pq": p, "at": a, "pi": pi})
    return in_maps


def kernel(attn, sims, sinds):
    from concourse.bass_utils import run_bass_kernel_spmd

    attn = np.asarray(attn, dtype=np.float32)
    sims = np.asarray(sims, dtype=np.float32)
    sinds = np.asarray(sinds)

    if "nc" not in _cached:
        _cached["nc"] = _build()
    nc = _cached["nc"]

    in_maps = _host_prep(attn, sims, sinds)
    res = run_bass_kernel_spmd(nc, in_maps, list(range(N_CORES)))

    out = np.empty((B, HD, H, W, K), dtype=np.float32)
    for cid in range(N_CORES):
        b, q = divmod(cid, 4)
        o = res.results[cid]["out"].reshape(H, WQ, HD, K).transpose(2, 0, 1, 3)
        out[b][:, :, WQ * q:WQ * (q + 1), :] = o
    return out


# revision 30
# speedup vs baseline: 1.1972x; 1.0073x over previous
import sys

if "/opt/trn_rl_repo" not in sys.path:
    sys.path.insert(0, "/opt/trn_rl_repo")

import numpy as np

B, HD, H, W, K = 2, 4, 128, 128, 49
KS = 7
NSP = 9
S = 64
N_CORES = 8
WQ = W // 4            # 32 columns per core
WB = 8                 # w-columns per block
NBLK = WQ // WB        # 4 blocks
G = WB * HD * NSP      # 288 (w,hd,s) groups per block
U_SZ = G * K           # 14112
AT_SZ = WB * HD * K    # 1568
ATQ = WQ * HD * K      # 6272

_cached = {}


def _build():
    import concourse.bass as bass
    import concourse.tile as tile
    from concourse import bacc, mybir

    f32 = mybir.dt.float32
    f16 = mybir.dt.float16

    nc = bacc.Bacc("TRN2", target_bir_lowering=False, debug=False, num_devices=N_CORES)
    pq = nc.dram_tensor("pq", [128, WQ * NSP * K], f16, kind="ExternalInput")
    at = nc.dram_tensor("at", [128, ATQ], f16, kind="ExternalInput")
    pi = nc.dram_tensor("pi", [128, WQ * NSP], f16, kind="ExternalInput")
    out_s = nc.dram_tensor("out", [128, ATQ], f32, kind="ExternalOutput")

    def ap(t, off, dims):
        return bass.AP(t, off, [list(d) for d in dims])

    def sap(tap, extra_off, dims):
        return bass.AP(tap.tensor, tap.offset + extra_off,
                       [list(tap.ap[0]), *[list(d) for d in dims]])

    mult = mybir.AluOpType.mult
    add = mybir.AluOpType.add
    HALF = U_SZ // 2

    with tile.TileContext(nc) as tc:
        with (
            tc.tile_pool(name="pip", bufs=1) as pip,
            tc.tile_pool(name="ep", bufs=1) as ep,
            tc.tile_pool(name="inp", bufs=2) as inp,
            tc.tile_pool(name="up", bufs=2) as up,
            tc.tile_pool(name="tp", bufs=1) as tp,
            tc.tile_pool(name="sp", bufs=2) as sp,
            tc.tile_pool(name="wp", bufs=2) as wp,
            tc.tile_pool(name="op", bufs=2) as op,
        ):
            pi_t = pip.tile([128, WQ * NSP], f16)
            at_t = ep.tile([128, ATQ], f16)
            e_t = ep.tile([128, ATQ], f16)
            nc.sync.dma_start(
                sap(at_t[:], 0, [(1, AT_SZ)]),
                ap(at, 0, [(ATQ, 128), (1, AT_SZ)]),
            )
            nc.sync.dma_start(pi_t[:], pi.ap())

            PH = WB * NSP * K // 2

            def load_p(blk):
                # split the P load across two HWDGE queues (sync + scalar)
                P4 = inp.tile([128, WB * NSP * K], f16, tag="P")
                nc.sync.dma_start(
                    sap(P4[:], 0, [(1, PH)]),
                    ap(pq, blk * WB * NSP * K, [(WQ * NSP * K, 128), (1, PH)]),
                )
                nc.scalar.dma_start(
                    sap(P4[:], PH, [(1, PH)]),
                    ap(pq, blk * WB * NSP * K + PH, [(WQ * NSP * K, 128), (1, PH)]),
                )
                return P4

            def front(blk, P4):
                # attn chunk for the NEXT block while this one computes
                if blk + 1 < NBLK:
                    nc.sync.dma_start(
                        sap(at_t[:], (blk + 1) * AT_SZ, [(1, AT_SZ)]),
                        ap(at, (blk + 1) * AT_SZ, [(ATQ, 128), (1, AT_SZ)]),
                    )
                # e = exp(attn) for this block (scalar engine); block 0 runs
                # in halves so the first u-TT starts sooner after the ramp
                n_exp = 2 if blk == 0 else 1
                ECH = AT_SZ // n_exp
                for c in range(n_exp):
                    nc.scalar.activation(
                        sap(e_t[:], blk * AT_SZ + c * ECH, [(1, ECH)]),
                        sap(at_t[:], blk * AT_SZ + c * ECH, [(1, ECH)]),
                        mybir.ActivationFunctionType.Exp,
                    )

                # u[w,hd,s,j] = P[w,s,j] * e[w,hd,j]   (fp16 2x)
                U = up.tile([128, U_SZ], f16, tag="U")
                n_u = 2 if blk == 0 else 1
                WBC = WB // n_u
                for c in range(n_u):
                    nc.vector.tensor_tensor(
                        out=sap(U[:], c * WBC * HD * NSP * K,
                                [(HD * NSP * K, WBC), (NSP * K, HD), (K, NSP), (1, K)]),
                        in0=sap(P4[:], c * WBC * NSP * K,
                                [(NSP * K, WBC), (0, HD), (K, NSP), (1, K)]),
                        in1=sap(e_t[:], blk * AT_SZ + c * WBC * HD * K,
                                [(HD * K, WBC), (K, HD), (0, NSP), (1, K)]),
                        op=mult,
                    )

                # den[w,hd,s] = sum_j u : fp16 halving tree then 4-wide reduce
                T24 = tp.tile([128, G * 24], f16, tag="T")
                nc.vector.tensor_tensor(
                    out=sap(T24[:], 0, [(24, G), (1, 24)]),
                    in0=sap(U[:], 0, [(K, G), (1, 24)]),
                    in1=sap(U[:], 24, [(K, G), (1, 24)]),
                    op=add,
                )
                nc.vector.tensor_tensor(
                    out=sap(T24[:], 0, [(24, G), (1, 12)]),
                    in0=sap(T24[:], 0, [(24, G), (1, 12)]),
                    in1=sap(T24[:], 12, [(24, G), (1, 12)]),
                    op=add,
                )
                nc.vector.tensor_tensor(
                    out=sap(T24[:], 0, [(24, G), (1, 6)]),
                    in0=sap(T24[:], 0, [(24, G), (1, 6)]),
                    in1=sap(T24[:], 6, [(24, G), (1, 6)]),
                    op=add,
                )
                D6 = sp.tile([128, G], f32, tag="D6")
                nc.vector.reduce_sum(
                    out=sap(D6[:], 0, [(1, G)]),
                    in_=sap(T24[:], 0, [(24, G), (1, 6)]),
                    axis=mybir.AxisListType.X,
                )
                # straggler u[...,48] added during den assembly (contig out)
                DEN = sp.tile([128, G], f32, tag="D")
                nc.vector.tensor_tensor(
                    out=sap(DEN[:], 0, [(1, G)]),
                    in0=sap(D6[:], 0, [(1, G)]),
                    in1=sap(U[:], 48, [(K, G)]),
                    op=add,
                )

                # wgt[w,hd,s] = pi[w,s] / den
                RCP = sp.tile([128, G], f32, tag="C")
                nc.vector.reciprocal_approx_fast(RCP[:], DEN[:])
                WT = sp.tile([128, G], f32, tag="W")
                nc.vector.tensor_tensor(
                    out=sap(WT[:], 0, [(NSP * HD, WB), (NSP, HD), (1, NSP)]),
                    in0=sap(RCP[:], 0, [(NSP * HD, WB), (NSP, HD), (1, NSP)]),
                    in1=sap(pi_t[:], blk * WB * NSP, [(NSP, WB), (0, HD), (1, NSP)]),
                    op=mult,
                )

                # broadcast wgt 7x on the scalar engine (49 = 7*7: the acc AP
                # re-reads each 7-wide run 7 times, staying 2x-packed while
                # the Act copy is 1/7 the work of a full 49-wide broadcast)
                WR = wp.tile([128, G * 7], f16, tag="WR")
                nc.scalar.copy(
                    sap(WR[:], 0, [(7, G), (1, 7)]),
                    sap(WT[:], 0, [(1, G), (0, 7)]),
                )
                return U, WR

            def back(blk, U, WR):
                # acc = u * wgt (in place, fp16 2x)
                nc.vector.tensor_tensor(
                    out=sap(U[:], 0, [(K, G), (7, 7), (1, 7)]),
                    in0=sap(U[:], 0, [(K, G), (7, 7), (1, 7)]),
                    in1=sap(WR[:], 0, [(7, G), (0, 7), (1, 7)]),
                    op=mult,
                )

                # sum over s: 8 -> 4 -> 2 -> 1, straggler s=8 folded in final
                eng = nc.vector
                eng.tensor_tensor(
                    out=sap(U[:], 0, [(NSP * K, WB * HD), (K, 4), (1, K)]),
                    in0=sap(U[:], 0, [(NSP * K, WB * HD), (K, 4), (1, K)]),
                    in1=sap(U[:], 4 * K, [(NSP * K, WB * HD), (K, 4), (1, K)]),
                    op=add,
                )
                eng.tensor_tensor(
                    out=sap(U[:], 0, [(NSP * K, WB * HD), (K, 2), (1, K)]),
                    in0=sap(U[:], 0, [(NSP * K, WB * HD), (K, 2), (1, K)]),
                    in1=sap(U[:], 2 * K, [(NSP * K, WB * HD), (K, 2), (1, K)]),
                    op=add,
                )
                eng.tensor_tensor(
                    out=sap(U[:], 0, [(NSP * K, WB * HD), (1, K)]),
                    in0=sap(U[:], 0, [(NSP * K, WB * HD), (1, K)]),
                    in1=sap(U[:], K, [(NSP * K, WB * HD), (1, K)]),
                    op=add,
                )
                # final add (folds straggler s=8) + fp16->fp32 casting store
                # (gpsimd SWDGE); last block in halves to overlap the tail
                O4 = op.tile([128, AT_SZ], f16, tag="O")
                n_fin = 2 if blk == NBLK - 1 else 1
                GF = WB * HD // n_fin
                for c in range(n_fin):
                    eng.tensor_tensor(
                        out=sap(O4[:], c * GF * K, [(K, GF), (1, K)]),
                        in0=sap(U[:], c * GF * NSP * K, [(NSP * K, GF), (1, K)]),
                        in1=sap(U[:], c * GF * NSP * K + 8 * K, [(NSP * K, GF), (1, K)]),
                        op=add,
                    )
                    nc.gpsimd.dma_start(
                        ap(out_s, blk * AT_SZ + c * GF * K, [(ATQ, 128), (1, GF * K)]),
                        sap(O4[:], c * GF * K, [(1, GF * K)]),
                    )

            for blk in range(NBLK):
                P4 = load_p(blk)
                U, WR = front(blk, P4)
                back(blk, U, WR)
    nc.compile()
    return nc


def _host_prep(attn, sims, sinds):
    hj = (np.clip(np.arange(H) - KS // 2, 0, H - KS)[:, None] + np.arange(KS)[None, :])
    wj = (np.clip(np.arange(W) - KS // 2, 0, W - KS)[:, None] + np.arange(KS)[None, :])
    h_idx = np.arange(H, dtype=np.intp)[:, None, None]
    w_idx = np.arange(WQ, dtype=np.intp)[None, :, None]
    in_maps = []
    for b in range(B):
        sims_b = sims[b]                                  # (S,H,W)
        SW = sims_b[:, hj, :]                             # (S,H,7,W)
        for q in range(4):
            wsl = slice(WQ * q, WQ * (q + 1))
            wq = wj[wsl]                                  # (WQ,7)
            SWq = np.ascontiguousarray(SW[:, :, :, wq])   # (S,H,7,WQ,7)
            c_idx = sinds[b][:, wsl, :].astype(np.intp)   # (H,WQ,9)
            p = SWq[c_idx, h_idx, :, w_idx, :]            # (H,WQ,9,7,7)
            p = p.reshape(H, WQ * NSP * K).astype(np.float16)
            pi = sims_b[c_idx, h_idx, w_idx + WQ * q]     # (H,WQ,9)
            pi = pi.reshape(H, WQ * NSP).astype(np.float16)
            a = attn[b][:, :, wsl, :].transpose(1, 2, 0, 3)  # (H,WQ,HD,K)
            a = np.ascontiguousarray(a.reshape(H, ATQ)).astype(np.float16)
            in_maps.append({"pq": p, "at": a, "pi": pi})
    return in_maps


def kernel(attn, sims, sinds):
    from concourse.bass_utils import run_bass_kernel_spmd

    attn = np.asarray(attn, dtype=np.float32)
    sims = np.asarray(sims, dtype=np.float32)
    sinds = np.asarray(sinds)

    if "nc" not in _cached:
        _cached["nc"] = _build()
    nc = _cached["nc"]

    in_maps = _host_prep(attn, sims, sinds)
    res = run_bass_kernel_spmd(nc, in_maps, list(range(N_CORES)))

    out = np.empty((B, HD, H, W, K), dtype=np.float32)
    for cid in range(N_CORES):
        b, q = divmod(cid, 4)
        o = res.results[cid]["out"].reshape(H, WQ, HD, K).transpose(2, 0, 1, 3)
        out[b][:, :, WQ * q:WQ * (q + 1), :] = o
    return out
